# revision 58
# baseline (speedup 1.0000x reference)
"""GAT layer kernel for Trainium2 (8 NeuronCores, SPMD).

Math note: in the reference, the per-destination softmax weights are only
used through their *mean* over each destination's incoming edges -- and a
softmax sums to 1, so attn_w[i] = 1/deg[i] (0 if deg==0) exactly.  The
output therefore reduces to mean aggregation:

    out[i] = (1/deg[i]) * sum_{e: col[e]=i} (x[row[e]] @ Wv.T + bv)
           = (agg[i] @ Wv.T) / deg[i] + bv        (deg>0; 0 otherwise)
    agg[i] = sum_{e: col[e]=i} x[row[e]]

Device strategy (dst-node sharded, 49 windows of 128 dst nodes per core):
  - host sorts edges by (dst window, src half) and packs each window's
    edge list into T = T_LO + T_HI chunks of 128 slots.
  - x is stored in HBM as bf16 (halves gather bytes; rel-err budget 2e-2
    is ~50x above bf16 noise).  Each window issues FOUR dma_gather calls
    (lo/hi half x 2 chunk ranges; int16 indices, x split into two
    <32768-row halves), one per SWDGE queue: the Pool engine's four Q7
    core-pairs generate descriptors for the same window concurrently
    (descgen is the kernel's bottleneck at ~8-11ns/index/pair), and
    windows complete in-order so PE never builds a backlog.
  - pad slots carry idx -1: the Q7 desc-gen trims trailing negatives at
    runtime, so padding costs no descriptors or DMA bytes.  True counts
    are fed per-core via num_idxs_reg registers (batched reg_loads, and
    a no-sync total-order chain keeps Tile from reordering the stream).
  - per window one bulk DVE tensor_tensor is_equal with broadcast APs
    builds all T one-hots [128 edge, T*128 dst] at once (prebuilt up to
    16 windows ahead); TensorE accumulates aggT[din, dst] += Xchunk^T @
    onehot into PSUM (bf16 matmuls, f32 accumulate).
  - epilogue (PSUM->SBUF bf16 cast + recip scale on the Scalar engine,
    out matmul deferred 4 windows): out = (aggT^T @ WvT + deg x bv) *
    recip[dst].
"""

import os
import numpy as np

P = 128
NCORES = 8
N = 50000
XLO = 25088                   # rows in the low half of x (< 32768 for int16)
XHI = N - XLO
DIN = 128
DOUT = 128
WPC = 49                      # windows per core
NWIN = NCORES * WPC           # 392
NPAD = NWIN * P               # 50176
G = 1                         # windows per gather group

XG_BUFS = 12                  # gather-tile pipeline depth (windows in flight)

_last_exec_ns = None
_cache = {}


def _groups():
    out = []
    g0 = 0
    while g0 < WPC:
        out.append((g0, min(G, WPC - g0)))
        g0 += G
    return out


def _ensure_ntff_hook():
    """The agent image's ``antenv`` lacks ``axon_hooks``; provide the tiny
    get/set registry and register the ctypes NTFF hook so trace=True works."""
    import sys
    import types
    if "antenv.axon_hooks" in sys.modules:
        return
    try:
        import antenv
        mod = types.ModuleType("antenv.axon_hooks")
        _h = [None]
        mod.set_axon_ntff_profile_hook = lambda hook: _h.__setitem__(0, hook)
        mod.get_axon_ntff_profile_hook = lambda: _h[0]
        sys.modules["antenv.axon_hooks"] = mod
        antenv.axon_hooks = mod
        from trn_agent_boot.trn_boot import _ntff_profile_via_ctypes
        hook = _ntff_profile_via_ctypes("/opt/axon/libaxon_pjrt.so")
        if hook is not None:
            mod.set_axon_ntff_profile_hook(hook)
    except Exception:
        pass


NSEG = 4                      # sub-gathers per window (one per SWDGE queue)


def _segments(T_LO, T_HI):
    """Chunk ranges of the four per-window sub-gathers: (half, c0, c1).

    ceil split (5/4 chunks): measured FASTER than the count-balanced 4/5
    split despite a larger max segment — the two lightly-loaded queues
    finish early and the remaining pairs then run at lower Q7/SBUF
    contention, which beats equalized loads (A/B'd on HW, ~10-15us)."""
    sl = (T_LO + 1) // 2
    sh = (T_HI + 1) // 2
    return [(0, 0, sl), (0, sl, T_LO), (1, 0, sh), (1, sh, T_HI)]


def _offsets(T):
    """Column offsets of the packed [P, CW] f32 constant tensor.
    bf16 regions (wvt/bv/deg) occupy half-width f32 column spans."""
    o = {}
    o["cnt"] = 0                          # int32 true idx counts, WPC*NSEG
    o["idx16"] = o["cnt"] + WPC * NSEG    # int16 idx (wrapped), WPC*T*4 f32
    o["col"] = o["idx16"] + WPC * T * 4   # col_local f32, WPC*T cols
    o["rec"] = o["col"] + WPC * T         # recip, WPC cols
    o["wvt"] = o["rec"] + WPC             # Wv.T bf16, DOUT/2 f32 cols
    o["iota"] = o["wvt"] + DOUT // 2      # iota ramp 0..127, P cols
    o["bv"] = o["iota"] + P               # bv bf16 at partition 0
    o["deg"] = o["bv"] + DOUT // 2        # deg bf16 at partition 0
    o["CW"] = o["deg"] + WPC * P // 2
    return o


def _patch_qaware_dmasw_lanes():
    """Tile's DMASW semaphore-lane round-robin ignores dma_gather's
    queue_num, but a DMASW lane must only ever be fed from ONE SWDGE queue
    (per-queue FIFOs drain independently, so cross-queue sharing breaks the
    in-order wait accounting and CoreSim rejects it).  Partition the 8 lanes
    as 2 per queue: Pool-engine DMAs with queue_num q round-robin over lanes
    {2q, 2q+1}."""
    import concourse.tile_sem_assignment as tsa
    import concourse.mybir as mybir

    if getattr(tsa, "_gat_qaware_patch", False):
        return
    tsa._gat_qaware_patch = True
    orig = tsa.TileClockTick._assign_tick
    DMAInst = tsa.DMAInst

    def _assign_tick_qaware(self, inst):
        q = getattr(inst, "queue_num", None)
        if (
            q is not None
            and isinstance(inst, DMAInst)
            and inst.engine == mybir.EngineType.Pool
            and self.swdge_sem_count >= 8
        ):
            rr = getattr(self, "_gat_q_lane_rr", None)
            if rr is None:
                rr = self._gat_q_lane_rr = {}
            k = rr.get(q, 0)
            lanes_per_q = self.swdge_sem_count // 4
            self.next_sw_dma_idx = q * lanes_per_q + (k % lanes_per_q)
            rr[q] = k + 1
        return orig(self, inst)

    tsa.TileClockTick._assign_tick = _assign_tick_qaware


def _build(T, T_LO, T_HI):
    import concourse.bacc as bacc
    import concourse.mybir as mybir
    from concourse.tile import TileContext

    _patch_qaware_dmasw_lanes()

    f32 = mybir.dt.float32
    bf16 = mybir.dt.bfloat16
    i16 = mybir.dt.int16

    o = _offsets(T)
    CW = o["CW"]

    i32 = mybir.dt.int32

    nc = bacc.Bacc(None, target_bir_lowering=False, num_swdge_queues=4)
    xlo_d = nc.dram_tensor("xlo", [XLO, DIN], bf16, kind="ExternalInput")
    xhi_d = nc.dram_tensor("xhi", [XHI, DIN], bf16, kind="ExternalInput")
    # int32 (not f32): the idx16 region's -1 padding forms NaN bit patterns
    # that float-dtype DMA validation would reject
    const_d = nc.dram_tensor("const", [P, CW], i32, kind="ExternalInput")
    out_d = nc.dram_tensor("out", [WPC * P, DOUT], f32, kind="ExternalOutput")

    with TileContext(nc) as tc:
        with (
            tc.tile_pool(name="const", bufs=1) as cpool,
            tc.tile_pool(name="xg", bufs=XG_BUFS) as xgpool,
            tc.tile_pool(name="oh", bufs=16) as ohpool,
            tc.tile_pool(name="ep", bufs=7) as eppool,
            tc.tile_pool(name="ps", bufs=3, space="PSUM") as pspool,
            tc.tile_pool(name="po", bufs=4, space="PSUM") as popool,
            tc.tile_pool(name="wp", bufs=1, space="PSUM") as wpool,
        ):
            const_sb = cpool.tile([P, CW], i32)
            # cnt + first 8 windows' idx16 first so the first gathers start
            # almost immediately; everything else afterwards
            idx_split = o["idx16"] + 8 * T * 8
            nc.sync.dma_start(out=const_sb[:, 0:idx_split],
                              in_=const_d[:, 0:idx_split])
            nc.sync.dma_start(out=const_sb[:, idx_split:o["col"]],
                              in_=const_d[:, idx_split:o["col"]])
            nc.sync.dma_start(out=const_sb[:, o["col"]:],
                              in_=const_d[:, o["col"]:])

            cnt_sb = const_sb[:, o["cnt"]:o["idx16"]]
            idx16_sb = const_sb[:, o["idx16"]:o["col"]].bitcast(i16)
            col_sb = const_sb[:, o["col"]:o["col"] + WPC * T].bitcast(f32)
            rec_sb = const_sb[:, o["rec"]:o["rec"] + WPC].bitcast(f32)
            wvt_sb = const_sb[:, o["wvt"]:o["wvt"] + DOUT // 2].bitcast(bf16)
            iota_sb = const_sb[:, o["iota"]:o["iota"] + P].bitcast(f32)
            bv_sb = const_sb[0:1, o["bv"]:o["bv"] + DOUT // 2].bitcast(bf16)
            deg_sb = const_sb[0:1, o["deg"]:o["deg"] + WPC * P // 2].bitcast(bf16)

            warm_ps = wpool.tile([1, 1], f32, tag="warm")
            # PE observes the const-load semaphore once
            cw0 = const_sb[0:1, 0:1].bitcast(f32)
            nc.tensor.matmul(out=warm_ps[:], lhsT=cw0,
                             rhs=cw0, start=True, stop=True)

            EPI_DEFER = 4
            pending = []

            def _epilogue(w, aggT_sb):
                out_ps = popool.tile([P, DOUT], f32, tag="outp")
                nc.tensor.matmul(out=out_ps[:], lhsT=aggT_sb[:],
                                 rhs=wvt_sb[:], start=True, stop=False)
                nc.tensor.matmul(out=out_ps[:],
                                 lhsT=deg_sb[0:1, w * P:(w + 1) * P],
                                 rhs=bv_sb[0:1, :], start=False, stop=True)
                out_sb = eppool.tile([P, DOUT], f32, tag="outs")
                # DVE, not ACT: the Scalar engine pays ~920ns per
                # EVENT_SEMAPHORE (vs ~85ns on DVE), so every epilogue hop
                # through ACT costs ~1us of sem processing
                nc.vector.tensor_scalar(
                    out=out_sb[:],
                    in0=out_ps[:],
                    scalar1=rec_sb[:, w:w + 1],
                    scalar2=None,
                    op0=mybir.AluOpType.mult,
                )
                nc.sync.dma_start(out=out_d[w * P:(w + 1) * P, :],
                                  in_=out_sb[:])

            segs = _segments(T_LO, T_HI)
            # 4 windows per register bank: one batched TENSOR_LOAD fills 16
            # count registers, keeping the Pool issue queue packed with
            # gathers (in-flight depth across the 4 Q7 pairs)
            RLW = 4
            cregs = [nc.gpsimd.alloc_register(f"cntreg{q}")
                     for q in range(RLW * NSEG)]
            # Tile's scheduler doesn't track register data deps, and the
            # emitted Pool-stream ORDER determines both register-read
            # correctness (num_idxs_reg resolves at NX decode, in issue
            # order) and Q7-pair overlap (adjacent instructions must hit
            # different queues).  Freeze the whole stream with a
            # total-order no-sync chain: rl -> g(q0) -> g(q1) -> ...
            from concourse.instruction_name_ordered_set import (
                InstructionNameOrderedSet)
            chain_prev = [None]

            def _chain(inst):
                if chain_prev[0] is not None:
                    deps = InstructionNameOrderedSet()
                    deps.add(chain_prev[0].ins.name)
                    inst.ins.add_nosync_dependencies_from(deps)
                chain_prev[0] = inst

            def _load_counts(w0, nwin):
                regs = cregs[:nwin * NSEG]
                rl = nc.gpsimd.reg_load(
                    regs, cnt_sb[0:1, w0 * NSEG:(w0 + nwin) * NSEG])
                _chain(rl)

            def _chain_gather(g):
                _chain(g)

            # DVE stream layout is load-bearing (the engine runs in order):
            # xg first-use memsets first, then OH_AHEAD prebuilt one-hots,
            # then per window [cast(w), scale(w-4), TT(w+OH_AHEAD)] -- so
            # one-hot prebuilding is never stuck behind a gather-paced cast
            # (which is what made the last windows' one-hots run in the
            # tail and pace it at ~3us/window).
            OH_AHEAD = 16
            xg_tiles = []
            for _xgi in range(WPC):
                xg_t = xgpool.tile([P, T * P], bf16, tag="xg")
                xg_tiles.append(xg_t)
            for sl in range(min(XG_BUFS, WPC)):
                # zero each slot's first use: trailing-trimmed gathers
                # leave pad slots holding whatever SBUF held before, and
                # NaN bit patterns would poison 0-weighted matmuls
                nc.vector.memset(xg_tiles[sl][:], 0)

            oh_tiles = {}

            def _emit_oh(w):
                oh = ohpool.tile([P, T * P], bf16, tag="oh")
                oh3 = oh[:].rearrange("p (t j) -> p t j", j=P)
                iota_b = iota_sb[:, :].unsqueeze(1).to_broadcast((P, T, P))
                col_b = col_sb[:, w * T:(w + 1) * T].unsqueeze(2) \
                    .to_broadcast((P, T, P))
                nc.vector.tensor_tensor(
                    out=oh3, in0=iota_b, in1=col_b,
                    op=mybir.AluOpType.is_equal,
                )
                oh_tiles[w] = oh

            for w in range(min(OH_AHEAD, WPC)):
                _emit_oh(w)

            goff16 = 0
            for gidx, (g0, Gg) in enumerate(_groups()):
                # every window splits into NSEG sub-gathers, one per SWDGE
                # queue: all four Q7 core-pairs work the same window in
                # lock-step, so windows complete in order and PE never
                # builds a backlog
                w = g0
                xg = xg_tiles[gidx]
                xg3 = xg[:].rearrange("p (c e) -> p c e", e=P)
                if gidx % RLW == 0:
                    _load_counts(w, min(RLW, WPC - w))
                for s, (half, c0, c1) in enumerate(segs):
                    ni = (c1 - c0) * P
                    cbase = 0 if half == 0 else T_LO
                    src = xlo_d if half == 0 else xhi_d
                    g = nc.gpsimd.dma_gather(
                        out_ap=xg3[:, cbase + c0:cbase + c1, :],
                        in_ap=src[:, :],
                        idxs_ap=idx16_sb[:, goff16:goff16 + ni // 16],
                        num_idxs=ni,
                        num_idxs_reg=cregs[(gidx % RLW) * NSEG + s],
                        elem_size=DIN,
                        single_packet=False,
                        queue_num=s,
                    )
                    _chain_gather(g)
                    goff16 += ni // 16
                warm_ps = wpool.tile([1, 1], f32, tag="warm")
                # PE observes the gather completions here
                nc.tensor.matmul(out=warm_ps[:], lhsT=xg[0:1, 0:1],
                                 rhs=xg[0:1, 0:1], start=True, stop=True)
                for wl in range(Gg):
                    w = g0 + wl
                    oh = oh_tiles[w]
                    agg_ps = pspool.tile([P, P], f32, tag="agg")
                    for t in range(T):
                        if t < T_LO:
                            c = wl * T_LO + t
                        else:
                            c = Gg * T_LO + wl * T_HI + (t - T_LO)
                        nc.tensor.matmul(
                            out=agg_ps[:],
                            lhsT=xg[:, c * P:(c + 1) * P],
                            rhs=oh[:, t * P:(t + 1) * P],
                            start=(t == 0),
                            stop=(t == T - 1),
                        )
                    # the first epilogue half (PSUM->SBUF cast, on DVE) can
                    # chase the accumulation immediately
                    aggT_sb = eppool.tile([P, P], bf16, tag="aggT")
                    nc.vector.tensor_copy(out=aggT_sb[:], in_=agg_ps[:])
                    # defer the PE half of the epilogue by EPI_DEFER windows
                    # so PE never blocks on the cast round trip
                    pending.append((w, aggT_sb))
                    if len(pending) > EPI_DEFER:
                        _epilogue(*pending.pop(0))
                    if w + OH_AHEAD < WPC:
                        _emit_oh(w + OH_AHEAD)
            for args in pending:
                _epilogue(*args)
    nc.compile()
    return nc


def _prep(x, row, col):
    """Host-side packing. Returns (T, T_LO, T_HI, per-core arrays)."""
    row = row.astype(np.int64)
    col = col.astype(np.int64)
    E = len(row)
    ishi = (row >= XLO).astype(np.int64)
    key = ((col >> 7) << 1) | ishi
    order = np.argsort(key, kind="stable")
    srow = row[order]
    scol = col[order]
    skey = key[order]

    deg = np.bincount(col, minlength=NPAD).astype(np.float32)
    recip = np.where(deg > 0, 1.0 / np.maximum(deg, 1.0), 0.0).astype(np.float32)

    cnt = np.bincount(key, minlength=2 * NWIN)
    lo_cnt, hi_cnt = cnt[0::2], cnt[1::2]
    T_LO = int(np.ceil(lo_cnt.max() / P))
    T_HI = int(np.ceil(hi_cnt.max() / P))
    T = T_LO + T_HI

    gstart = np.zeros(2 * NWIN + 1, np.int64)
    np.cumsum(cnt, out=gstart[1:])
    epos = np.arange(E, dtype=np.int64) - gstart[skey]
    p = epos % P
    tw = epos // P
    whalf = skey & 1
    win = skey >> 1
    tchunk = np.where(whalf == 1, tw + T_LO, tw)

    col_arr = np.full((NWIN, P, T), -1.0, np.float32)
    col_arr[win, p, tchunk] = (scol & (P - 1)).astype(np.float32)

    # padding slots get idx -1: the Q7 desc-gen kernel trims trailing
    # negative indices at runtime (per core), skipping their descriptors
    # and DMA bytes.  Their one-hot cols are -1 so the stale SBUF data in
    # those slots never contributes to the matmul.
    pad = np.int16(0) if os.environ.get("GAT_SIM_NOTRIM") else np.int16(-1)
    idx_lo = np.full((NWIN, T_LO * P), pad, np.int16)
    idx_hi = np.full((NWIN, T_HI * P), pad, np.int16)
    lo_m = whalf == 0
    hi_m = whalf == 1
    idx_lo[win[lo_m], epos[lo_m]] = srow[lo_m].astype(np.int16)
    idx_hi[win[hi_m], epos[hi_m]] = (srow[hi_m] - XLO).astype(np.int16)

    segs = _segments(T_LO, T_HI)
    per_core = []
    for c in range(NCORES):
        wsl = slice(c * WPC, (c + 1) * WPC)
        # wrapped idx16 layout: per sub-gather, index i at [i%16, i//16],
        # replicated across the 8 groups of 16 partitions.  True counts
        # (non-pad indices) per sub-gather feed num_idxs_reg so the Q7
        # desc-gen's trailing-negative trim matches the decode bookkeeping.
        cols16 = []
        cnts = np.zeros(WPC * NSEG, np.int32)
        for wl in range(WPC):
            wabs = c * WPC + wl
            for s, (half, c0, c1) in enumerate(segs):
                arr = idx_lo if half == 0 else idx_hi
                havecnt = int(lo_cnt[wabs] if half == 0 else hi_cnt[wabs])
                flat = arr[wabs, c0 * P:c1 * P].copy()
                if os.environ.get("GAT_SIM_NOTRIM"):
                    cnt_s = (c1 - c0) * P    # pads are idx 0: all "valid"
                else:
                    cnt_s = min(max(havecnt - c0 * P, 0), (c1 - c0) * P)
                    if cnt_s == 0:
                        flat[0] = 0      # keep >=1 valid idx per sub-gather
                        cnt_s = 1
                cnts[wl * NSEG + s] = cnt_s
                wrapped = flat.reshape(-1, 16).T             # [16, ni/16]
                cols16.append(np.tile(wrapped, (8, 1)))      # [128, ni/16]
        idx16_map = np.concatenate(cols16, axis=1)           # [128, WPC*T*8]
        col_map = np.ascontiguousarray(
            col_arr[wsl].transpose(1, 0, 2).reshape(P, WPC * T))
        rec_map = np.ascontiguousarray(
            recip[c * WPC * P:(c + 1) * WPC * P].reshape(WPC, P).T)
        deg_map = np.ascontiguousarray(
            deg[c * WPC * P:(c + 1) * WPC * P].reshape(1, WPC * P))
        per_core.append((idx16_map, col_map, rec_map, deg_map, cnts))
    return T, T_LO, T_HI, per_core


def _pack_const(T, idx16_map, col_map, rec_map, deg_map, cnts, wvt_bf, bv_bf):
    """Pack the [P, CW] f32 const tensor.  bf16 payloads (wvt/bv/deg) are
    written through a uint16 view at doubled column offsets."""
    o = _offsets(T)
    arr = np.zeros((P, o["CW"]), np.float32)
    u16 = arr.view(np.uint16)
    assert idx16_map.shape == (P, WPC * T * 8)
    arr[0:1, o["cnt"]:o["idx16"]] = cnts.astype(np.int32).view(np.float32)
    arr[:, o["idx16"]:o["col"]] = idx16_map.view(np.float32)
    arr[:, o["col"]:o["col"] + WPC * T] = col_map
    arr[:, o["rec"]:o["rec"] + WPC] = rec_map
    u16[:, 2 * o["wvt"]:2 * o["wvt"] + DOUT] = wvt_bf.view(np.uint16)
    arr[:, o["iota"]:o["iota"] + P] = np.arange(P, dtype=np.float32)[None, :]
    u16[0, 2 * o["bv"]:2 * o["bv"] + DOUT] = bv_bf.view(np.uint16).ravel()
    u16[0, 2 * o["deg"]:2 * o["deg"] + WPC * P] = \
        deg_map.astype(wvt_bf.dtype).view(np.uint16).ravel()
    return arr


def _device_inputs(inputs):
    """Host prep shared by kernel() and the sim harness.
    Returns (T, T_LO, T_HI, in_maps)."""
    import concourse.mybir as mybir
    bf16 = mybir.dt.np(mybir.dt.bfloat16)

    x = np.ascontiguousarray(np.asarray(inputs["x"], dtype=np.float32))
    ei = np.asarray(inputs["edge_index"])
    row = np.asarray(ei[0]).astype(np.int64)
    col = np.asarray(ei[1]).astype(np.int64)
    Wv = np.asarray(inputs["Wv"], dtype=np.float32)
    bv = np.asarray(inputs["bv"], dtype=np.float32)

    wvt_bf = np.ascontiguousarray(Wv.T.astype(bf16))       # [DIN, DOUT] bf16
    bv_bf = np.ascontiguousarray(bv.reshape(1, DOUT).astype(bf16))

    T, T_LO, T_HI, per_core = _prep(x, row, col)

    xbf = x.astype(bf16)
    xlo = np.ascontiguousarray(xbf[:XLO])
    xhi = np.ascontiguousarray(xbf[XLO:])
    in_maps = []
    for c in range(NCORES):
        const = _pack_const(T, *per_core[c], wvt_bf, bv_bf)
        in_maps.append({"xlo": xlo, "xhi": xhi,
                        "const": const.view(np.int32)})
    return T, T_LO, T_HI, in_maps


def kernel(**inputs):
    global _last_exec_ns
    _ensure_ntff_hook()
    from concourse.bass_utils import run_bass_kernel_spmd

    T, T_LO, T_HI, in_maps = _device_inputs(inputs)

    key = (T, T_LO, T_HI)
    if key not in _cache:
        _cache[key] = _build(T, T_LO, T_HI)
    nc = _cache[key]

    trace = bool(os.environ.get("GAT_TRACE"))
    res = run_bass_kernel_spmd(nc, in_maps, list(range(NCORES)), trace=trace)
    _last_exec_ns = res.exec_time_ns
    globals()["_last_res"] = res

    out = np.concatenate([res.results[c]["out"] for c in range(NCORES)], axis=0)
    return np.ascontiguousarray(out[:N])


# revision 65
# speedup vs baseline: 1.2279x; 1.2279x over previous
"""GAT layer kernel for Trainium2 (8 NeuronCores, SPMD).

Math note: in the reference, the per-destination softmax weights are only
used through their *mean* over each destination's incoming edges -- and a
softmax sums to 1, so attn_w[i] = 1/deg[i] (0 if deg==0) exactly.  The
output therefore reduces to mean aggregation:

    out[i] = (1/deg[i]) * sum_{e: col[e]=i} (x[row[e]] @ Wv.T + bv)
           = (agg[i] @ Wv.T) / deg[i] + bv        (deg>0; 0 otherwise)
    agg[i] = sum_{e: col[e]=i} x[row[e]]

Device strategy (dst-node sharded, 49 windows of 128 dst nodes per core):
  - host sorts edges by (dst window, src half) and packs each window's
    edge list into T = T_LO + T_HI chunks of 128 slots.
  - x is stored in HBM as bf16 (halves gather bytes; rel-err budget 2e-2
    is ~50x above bf16 noise).  Each window issues FOUR dma_gather calls
    (lo/hi half x 2 chunk ranges; int16 indices, x split into two
    <32768-row halves), one per SWDGE queue: the Pool engine's four Q7
    core-pairs generate descriptors for the same window concurrently
    (descgen is the kernel's bottleneck at ~8-11ns/index/pair), and
    windows complete in-order so PE never builds a backlog.
  - pad slots carry idx -1: the Q7 desc-gen trims trailing negatives at
    runtime, so padding costs no descriptors or DMA bytes.  True counts
    are fed per-core via num_idxs_reg registers (batched reg_loads, and
    a no-sync total-order chain keeps Tile from reordering the stream).
  - per window one bulk DVE tensor_tensor is_equal with broadcast APs
    builds all T one-hots [128 edge, T*128 dst] at once (prebuilt up to
    16 windows ahead); TensorE accumulates aggT[din, dst] += Xchunk^T @
    onehot into PSUM (bf16 matmuls, f32 accumulate).
  - epilogue (PSUM->SBUF bf16 cast + recip scale on the Scalar engine,
    out matmul deferred 4 windows): out = (aggT^T @ WvT + deg x bv) *
    recip[dst].
"""

import os
import numpy as np

P = 128
NCORES = 8
N = 50000
XLO = 25088                   # rows in the low half of x (< 32768 for int16)
XHI = N - XLO
DIN = 128
DOUT = 128
WPC = 49                      # windows per core
NWIN = NCORES * WPC           # 392
NPAD = NWIN * P               # 50176
G = 1                         # windows per gather group

XG_BUFS = 12                  # gather-tile pipeline depth (windows in flight)

_last_exec_ns = None
_cache = {}


def _groups():
    out = []
    g0 = 0
    while g0 < WPC:
        out.append((g0, min(G, WPC - g0)))
        g0 += G
    return out


def _ensure_ntff_hook():
    """The agent image's ``antenv`` lacks ``axon_hooks``; provide the tiny
    get/set registry and register the ctypes NTFF hook so trace=True works."""
    import sys
    import types
    if "antenv.axon_hooks" in sys.modules:
        return
    try:
        import antenv
        mod = types.ModuleType("antenv.axon_hooks")
        _h = [None]
        mod.set_axon_ntff_profile_hook = lambda hook: _h.__setitem__(0, hook)
        mod.get_axon_ntff_profile_hook = lambda: _h[0]
        sys.modules["antenv.axon_hooks"] = mod
        antenv.axon_hooks = mod
        from trn_agent_boot.trn_boot import _ntff_profile_via_ctypes
        hook = _ntff_profile_via_ctypes("/opt/axon/libaxon_pjrt.so")
        if hook is not None:
            mod.set_axon_ntff_profile_hook(hook)
    except Exception:
        pass


NSEG = 4                      # sub-gathers per window (one per SWDGE queue)


def _segments(T_LO, T_HI):
    """Chunk ranges of the four per-window sub-gathers: (half, c0, c1).

    ceil split (5/4 chunks): measured FASTER than the count-balanced 4/5
    split despite a larger max segment — the two lightly-loaded queues
    finish early and the remaining pairs then run at lower Q7/SBUF
    contention, which beats equalized loads (A/B'd on HW, ~10-15us)."""
    sl = (T_LO + 1) // 2
    sh = (T_HI + 1) // 2
    return [(0, 0, sl), (0, sl, T_LO), (1, 0, sh), (1, sh, T_HI)]


def _offsets(T):
    """Column offsets of the packed [P, CW] f32 constant tensor.
    bf16 regions (wvt/bv/deg) occupy half-width f32 column spans."""
    o = {}
    o["cnt"] = 0                          # int32 true idx counts, WPC*NSEG
    o["idx16"] = o["cnt"] + WPC * NSEG    # int16 idx (wrapped), WPC*T*4 f32
    o["col"] = o["idx16"] + WPC * T * 4   # col_local f32, WPC*T cols
    o["rec"] = o["col"] + WPC * T         # recip, WPC cols
    o["wvt"] = o["rec"] + WPC             # Wv.T bf16, DOUT/2 f32 cols
    o["iota"] = o["wvt"] + DOUT // 2      # iota ramp 0..127, P cols
    o["bv"] = o["iota"] + P               # bv bf16 at partition 0
    o["deg"] = o["bv"] + DOUT // 2        # deg bf16 at partition 0
    o["CW"] = o["deg"] + WPC * P // 2
    return o


def _patch_qaware_dmasw_lanes():
    """Tile's DMASW semaphore-lane round-robin ignores dma_gather's
    queue_num, but a DMASW lane must only ever be fed from ONE SWDGE queue
    (per-queue FIFOs drain independently, so cross-queue sharing breaks the
    in-order wait accounting and CoreSim rejects it).  Partition the 8 lanes
    as 2 per queue: Pool-engine DMAs with queue_num q round-robin over lanes
    {2q, 2q+1}."""
    import concourse.tile_sem_assignment as tsa
    import concourse.mybir as mybir

    if getattr(tsa, "_gat_qaware_patch", False):
        return
    tsa._gat_qaware_patch = True
    orig = tsa.TileClockTick._assign_tick
    DMAInst = tsa.DMAInst

    def _assign_tick_qaware(self, inst):
        q = getattr(inst, "queue_num", None)
        if (
            q is not None
            and isinstance(inst, DMAInst)
            and inst.engine == mybir.EngineType.Pool
            and self.swdge_sem_count >= 8
        ):
            rr = getattr(self, "_gat_q_lane_rr", None)
            if rr is None:
                rr = self._gat_q_lane_rr = {}
            k = rr.get(q, 0)
            lanes_per_q = self.swdge_sem_count // 4
            self.next_sw_dma_idx = q * lanes_per_q + (k % lanes_per_q)
            rr[q] = k + 1
        return orig(self, inst)

    tsa.TileClockTick._assign_tick = _assign_tick_qaware


def _build(T, T_LO, T_HI):
    import concourse.bacc as bacc
    import concourse.mybir as mybir
    from concourse.tile import TileContext

    _patch_qaware_dmasw_lanes()

    f32 = mybir.dt.float32
    bf16 = mybir.dt.bfloat16
    i16 = mybir.dt.int16

    o = _offsets(T)
    CW = o["CW"]

    i32 = mybir.dt.int32

    nc = bacc.Bacc(None, target_bir_lowering=False, num_swdge_queues=4)
    xlo_d = nc.dram_tensor("xlo", [XLO, DIN], bf16, kind="ExternalInput")
    xhi_d = nc.dram_tensor("xhi", [XHI, DIN], bf16, kind="ExternalInput")
    # int32 (not f32): the idx16 region's -1 padding forms NaN bit patterns
    # that float-dtype DMA validation would reject
    const_d = nc.dram_tensor("const", [P, CW], i32, kind="ExternalInput")
    out_d = nc.dram_tensor("out", [WPC * P, DOUT], f32, kind="ExternalOutput")

    with TileContext(nc) as tc:
        with (
            tc.tile_pool(name="const", bufs=1) as cpool,
            tc.tile_pool(name="xg", bufs=XG_BUFS) as xgpool,
            tc.tile_pool(name="oh", bufs=16) as ohpool,
            tc.tile_pool(name="ep", bufs=7) as eppool,
            tc.tile_pool(name="ps", bufs=3, space="PSUM") as pspool,
            tc.tile_pool(name="po", bufs=4, space="PSUM") as popool,
            tc.tile_pool(name="wp", bufs=1, space="PSUM") as wpool,
        ):
            const_sb = cpool.tile([P, CW], i32)
            # cnt + first 8 windows' idx16 first so the first gathers start
            # almost immediately; everything else afterwards
            idx_split = o["idx16"] + 8 * T * 8
            nc.sync.dma_start(out=const_sb[:, 0:idx_split],
                              in_=const_d[:, 0:idx_split])
            nc.sync.dma_start(out=const_sb[:, idx_split:o["col"]],
                              in_=const_d[:, idx_split:o["col"]])
            nc.sync.dma_start(out=const_sb[:, o["col"]:],
                              in_=const_d[:, o["col"]:])

            cnt_sb = const_sb[:, o["cnt"]:o["idx16"]]
            idx16_sb = const_sb[:, o["idx16"]:o["col"]].bitcast(i16)
            col_sb = const_sb[:, o["col"]:o["col"] + WPC * T].bitcast(f32)
            rec_sb = const_sb[:, o["rec"]:o["rec"] + WPC].bitcast(f32)
            wvt_sb = const_sb[:, o["wvt"]:o["wvt"] + DOUT // 2].bitcast(bf16)
            iota_sb = const_sb[:, o["iota"]:o["iota"] + P].bitcast(f32)
            bv_sb = const_sb[0:1, o["bv"]:o["bv"] + DOUT // 2].bitcast(bf16)
            deg_sb = const_sb[0:1, o["deg"]:o["deg"] + WPC * P // 2].bitcast(bf16)

            warm_ps = wpool.tile([1, 1], f32, tag="warm")
            # PE observes the const-load semaphore once
            cw0 = const_sb[0:1, 0:1].bitcast(f32)
            nc.tensor.matmul(out=warm_ps[:], lhsT=cw0,
                             rhs=cw0, start=True, stop=True)

            EPI_DEFER = 4
            pending = []

            def _epilogue(w, aggT_sb):
                out_ps = popool.tile([P, DOUT], f32, tag="outp")
                nc.tensor.matmul(out=out_ps[:], lhsT=aggT_sb[:],
                                 rhs=wvt_sb[:], start=True, stop=False)
                nc.tensor.matmul(out=out_ps[:],
                                 lhsT=deg_sb[0:1, w * P:(w + 1) * P],
                                 rhs=bv_sb[0:1, :], start=False, stop=True)
                out_sb = eppool.tile([P, DOUT], f32, tag="outs")
                nc.vector.tensor_scalar(
                    out=out_sb[:],
                    in0=out_ps[:],
                    scalar1=rec_sb[:, w:w + 1],
                    scalar2=None,
                    op0=mybir.AluOpType.mult,
                )
                nc.sync.dma_start(out=out_d[w * P:(w + 1) * P, :],
                                  in_=out_sb[:])

            segs = _segments(T_LO, T_HI)
            # 4 windows per register bank: one batched TENSOR_LOAD fills 16
            # count registers, keeping the Pool issue queue packed with
            # gathers (in-flight depth across the 4 Q7 pairs)
            RLW = 4
            cregs = [nc.gpsimd.alloc_register(f"cntreg{q}")
                     for q in range(RLW * NSEG)]
            # Tile's scheduler doesn't track register data deps, and the
            # emitted Pool-stream ORDER determines both register-read
            # correctness (num_idxs_reg resolves at NX decode, in issue
            # order) and Q7-pair overlap (adjacent instructions must hit
            # different queues).  Freeze the whole stream with a
            # total-order no-sync chain: rl -> g(q0) -> g(q1) -> ...
            from concourse.instruction_name_ordered_set import (
                InstructionNameOrderedSet)
            chain_prev = [None]

            def _chain(inst):
                if chain_prev[0] is not None:
                    deps = InstructionNameOrderedSet()
                    deps.add(chain_prev[0].ins.name)
                    inst.ins.add_nosync_dependencies_from(deps)
                chain_prev[0] = inst

            def _load_counts(w0, nwin):
                regs = cregs[:nwin * NSEG]
                rl = nc.gpsimd.reg_load(
                    regs, cnt_sb[0:1, w0 * NSEG:(w0 + nwin) * NSEG])
                _chain(rl)

            def _chain_gather(g):
                _chain(g)

            goff16 = 0
            for gidx, (g0, Gg) in enumerate(_groups()):
                # every window splits into NSEG sub-gathers, one per SWDGE
                # queue: all four Q7 core-pairs work the same window in
                # lock-step, so windows complete in order and PE never
                # builds a backlog
                w = g0
                xg = xgpool.tile([P, Gg * T * P], bf16, tag="xg")
                if gidx < XG_BUFS:
                    # zero each slot's first use: trailing-trimmed gathers
                    # leave pad slots holding whatever SBUF held before, and
                    # NaN bit patterns would poison 0-weighted matmuls
                    nc.vector.memset(xg[:], 0)
                xg3 = xg[:].rearrange("p (c e) -> p c e", e=P)
                if gidx % RLW == 0:
                    _load_counts(w, min(RLW, WPC - w))
                for s, (half, c0, c1) in enumerate(segs):
                    ni = (c1 - c0) * P
                    cbase = 0 if half == 0 else T_LO
                    src = xlo_d if half == 0 else xhi_d
                    g = nc.gpsimd.dma_gather(
                        out_ap=xg3[:, cbase + c0:cbase + c1, :],
                        in_ap=src[:, :],
                        idxs_ap=idx16_sb[:, goff16:goff16 + ni // 16],
                        num_idxs=ni,
                        num_idxs_reg=cregs[(gidx % RLW) * NSEG + s],
                        elem_size=DIN,
                        single_packet=False,
                        queue_num=s,
                    )
                    _chain_gather(g)
                    goff16 += ni // 16
                warm_ps = wpool.tile([1, 1], f32, tag="warm")
                # PE observes the gather completions here
                nc.tensor.matmul(out=warm_ps[:], lhsT=xg[0:1, 0:1],
                                 rhs=xg[0:1, 0:1], start=True, stop=True)
                for wl in range(Gg):
                    w = g0 + wl
                    # bulk one-hot: oh[p, t*128+j] = (col[p, w*T+t] == j)
                    oh = ohpool.tile([P, T * P], bf16, tag="oh")
                    oh3 = oh[:].rearrange("p (t j) -> p t j", j=P)
                    iota_b = iota_sb[:, :].unsqueeze(1).to_broadcast((P, T, P))
                    col_b = col_sb[:, w * T:(w + 1) * T].unsqueeze(2) \
                        .to_broadcast((P, T, P))
                    nc.vector.tensor_tensor(
                        out=oh3, in0=iota_b, in1=col_b,
                        op=mybir.AluOpType.is_equal,
                    )
                    agg_ps = pspool.tile([P, P], f32, tag="agg")
                    for t in range(T):
                        if t < T_LO:
                            c = wl * T_LO + t
                        else:
                            c = Gg * T_LO + wl * T_HI + (t - T_LO)
                        nc.tensor.matmul(
                            out=agg_ps[:],
                            lhsT=xg[:, c * P:(c + 1) * P],
                            rhs=oh[:, t * P:(t + 1) * P],
                            start=(t == 0),
                            stop=(t == T - 1),
                        )
                    # the first epilogue half (PSUM->SBUF cast, on DVE: the
                    # ACT engine pays ~920ns per EVENT_SEMAPHORE vs ~85ns
                    # on DVE) can chase the accumulation immediately
                    aggT_sb = eppool.tile([P, P], bf16, tag="aggT")
                    nc.vector.tensor_copy(out=aggT_sb[:], in_=agg_ps[:])
                    # defer the PE half of the epilogue by EPI_DEFER windows
                    # so PE never blocks on the ACT round trip
                    pending.append((w, aggT_sb))
                    if len(pending) > EPI_DEFER:
                        _epilogue(*pending.pop(0))
            for args in pending:
                _epilogue(*args)
    nc.compile()
    return nc


def _prep(x, row, col):
    """Host-side packing. Returns (T, T_LO, T_HI, per-core arrays)."""
    row = row.astype(np.int64)
    col = col.astype(np.int64)
    E = len(row)
    ishi = (row >= XLO).astype(np.int64)
    key = ((col >> 7) << 1) | ishi
    order = np.argsort(key, kind="stable")
    srow = row[order]
    scol = col[order]
    skey = key[order]

    deg = np.bincount(col, minlength=NPAD).astype(np.float32)
    recip = np.where(deg > 0, 1.0 / np.maximum(deg, 1.0), 0.0).astype(np.float32)

    cnt = np.bincount(key, minlength=2 * NWIN)
    lo_cnt, hi_cnt = cnt[0::2], cnt[1::2]
    T_LO = int(np.ceil(lo_cnt.max() / P))
    T_HI = int(np.ceil(hi_cnt.max() / P))
    T = T_LO + T_HI

    gstart = np.zeros(2 * NWIN + 1, np.int64)
    np.cumsum(cnt, out=gstart[1:])
    epos = np.arange(E, dtype=np.int64) - gstart[skey]
    p = epos % P
    tw = epos // P
    whalf = skey & 1
    win = skey >> 1
    tchunk = np.where(whalf == 1, tw + T_LO, tw)

    col_arr = np.full((NWIN, P, T), -1.0, np.float32)
    col_arr[win, p, tchunk] = (scol & (P - 1)).astype(np.float32)

    # padding slots get idx -1: the Q7 desc-gen kernel trims trailing
    # negative indices at runtime (per core), skipping their descriptors
    # and DMA bytes.  Their one-hot cols are -1 so the stale SBUF data in
    # those slots never contributes to the matmul.
    pad = np.int16(0) if os.environ.get("GAT_SIM_NOTRIM") else np.int16(-1)
    idx_lo = np.full((NWIN, T_LO * P), pad, np.int16)
    idx_hi = np.full((NWIN, T_HI * P), pad, np.int16)
    lo_m = whalf == 0
    hi_m = whalf == 1
    idx_lo[win[lo_m], epos[lo_m]] = srow[lo_m].astype(np.int16)
    idx_hi[win[hi_m], epos[hi_m]] = (srow[hi_m] - XLO).astype(np.int16)

    segs = _segments(T_LO, T_HI)
    per_core = []
    for c in range(NCORES):
        wsl = slice(c * WPC, (c + 1) * WPC)
        # wrapped idx16 layout: per sub-gather, index i at [i%16, i//16],
        # replicated across the 8 groups of 16 partitions.  True counts
        # (non-pad indices) per sub-gather feed num_idxs_reg so the Q7
        # desc-gen's trailing-negative trim matches the decode bookkeeping.
        cols16 = []
        cnts = np.zeros(WPC * NSEG, np.int32)
        for wl in range(WPC):
            wabs = c * WPC + wl
            for s, (half, c0, c1) in enumerate(segs):
                arr = idx_lo if half == 0 else idx_hi
                havecnt = int(lo_cnt[wabs] if half == 0 else hi_cnt[wabs])
                flat = arr[wabs, c0 * P:c1 * P].copy()
                if os.environ.get("GAT_SIM_NOTRIM"):
                    cnt_s = (c1 - c0) * P    # pads are idx 0: all "valid"
                else:
                    cnt_s = min(max(havecnt - c0 * P, 0), (c1 - c0) * P)
                    if cnt_s == 0:
                        flat[0] = 0      # keep >=1 valid idx per sub-gather
                        cnt_s = 1
                cnts[wl * NSEG + s] = cnt_s
                wrapped = flat.reshape(-1, 16).T             # [16, ni/16]
                cols16.append(np.tile(wrapped, (8, 1)))      # [128, ni/16]
        idx16_map = np.concatenate(cols16, axis=1)           # [128, WPC*T*8]
        col_map = np.ascontiguousarray(
            col_arr[wsl].transpose(1, 0, 2).reshape(P, WPC * T))
        rec_map = np.ascontiguousarray(
            recip[c * WPC * P:(c + 1) * WPC * P].reshape(WPC, P).T)
        deg_map = np.ascontiguousarray(
            deg[c * WPC * P:(c + 1) * WPC * P].reshape(1, WPC * P))
        per_core.append((idx16_map, col_map, rec_map, deg_map, cnts))
    return T, T_LO, T_HI, per_core


def _pack_const(T, idx16_map, col_map, rec_map, deg_map, cnts, wvt_bf, bv_bf):
    """Pack the [P, CW] f32 const tensor.  bf16 payloads (wvt/bv/deg) are
    written through a uint16 view at doubled column offsets."""
    o = _offsets(T)
    arr = np.zeros((P, o["CW"]), np.float32)
    u16 = arr.view(np.uint16)
    assert idx16_map.shape == (P, WPC * T * 8)
    arr[0:1, o["cnt"]:o["idx16"]] = cnts.astype(np.int32).view(np.float32)
    arr[:, o["idx16"]:o["col"]] = idx16_map.view(np.float32)
    arr[:, o["col"]:o["col"] + WPC * T] = col_map
    arr[:, o["rec"]:o["rec"] + WPC] = rec_map
    u16[:, 2 * o["wvt"]:2 * o["wvt"] + DOUT] = wvt_bf.view(np.uint16)
    arr[:, o["iota"]:o["iota"] + P] = np.arange(P, dtype=np.float32)[None, :]
    u16[0, 2 * o["bv"]:2 * o["bv"] + DOUT] = bv_bf.view(np.uint16).ravel()
    u16[0, 2 * o["deg"]:2 * o["deg"] + WPC * P] = \
        deg_map.astype(wvt_bf.dtype).view(np.uint16).ravel()
    return arr


def _device_inputs(inputs):
    """Host prep shared by kernel() and the sim harness.
    Returns (T, T_LO, T_HI, in_maps)."""
    import concourse.mybir as mybir
    bf16 = mybir.dt.np(mybir.dt.bfloat16)

    x = np.ascontiguousarray(np.asarray(inputs["x"], dtype=np.float32))
    ei = np.asarray(inputs["edge_index"])
    row = np.asarray(ei[0]).astype(np.int64)
    col = np.asarray(ei[1]).astype(np.int64)
    Wv = np.asarray(inputs["Wv"], dtype=np.float32)
    bv = np.asarray(inputs["bv"], dtype=np.float32)

    wvt_bf = np.ascontiguousarray(Wv.T.astype(bf16))       # [DIN, DOUT] bf16
    bv_bf = np.ascontiguousarray(bv.reshape(1, DOUT).astype(bf16))

    T, T_LO, T_HI, per_core = _prep(x, row, col)

    xbf = x.astype(bf16)
    xlo = np.ascontiguousarray(xbf[:XLO])
    xhi = np.ascontiguousarray(xbf[XLO:])
    in_maps = []
    for c in range(NCORES):
        const = _pack_const(T, *per_core[c], wvt_bf, bv_bf)
        in_maps.append({"xlo": xlo, "xhi": xhi,
                        "const": const.view(np.int32)})
    return T, T_LO, T_HI, in_maps


def kernel(**inputs):
    global _last_exec_ns
    _ensure_ntff_hook()
    from concourse.bass_utils import run_bass_kernel_spmd

    T, T_LO, T_HI, in_maps = _device_inputs(inputs)

    key = (T, T_LO, T_HI)
    if key not in _cache:
        _cache[key] = _build(T, T_LO, T_HI)
    nc = _cache[key]

    trace = bool(os.environ.get("GAT_TRACE"))
    res = run_bass_kernel_spmd(nc, in_maps, list(range(NCORES)), trace=trace)
    _last_exec_ns = res.exec_time_ns
    globals()["_last_res"] = res

    out = np.concatenate([res.results[c]["out"] for c in range(NCORES)], axis=0)
    return np.ascontiguousarray(out[:N])


# revision 67
# speedup vs baseline: 1.4094x; 1.1478x over previous
"""GAT layer kernel for Trainium2 (8 NeuronCores, SPMD).

Math note: in the reference, the per-destination softmax weights are only
used through their *mean* over each destination's incoming edges -- and a
softmax sums to 1, so attn_w[i] = 1/deg[i] (0 if deg==0) exactly.  The
output therefore reduces to mean aggregation:

    out[i] = (1/deg[i]) * sum_{e: col[e]=i} (x[row[e]] @ Wv.T + bv)
           = (agg[i] @ Wv.T) / deg[i] + bv        (deg>0; 0 otherwise)
    agg[i] = sum_{e: col[e]=i} x[row[e]]

Device strategy (dst-node sharded, 49 windows of 128 dst nodes per core):
  - host sorts edges by (dst window, src half) and packs each window's
    edge list into T = T_LO + T_HI chunks of 128 slots.
  - x is stored in HBM as bf16 (halves gather bytes; rel-err budget 2e-2
    is ~50x above bf16 noise).  Each window issues FOUR dma_gather calls
    (lo/hi half x 2 chunk ranges; int16 indices, x split into two
    <32768-row halves), one per SWDGE queue: the Pool engine's four Q7
    core-pairs generate descriptors for the same window concurrently
    (descgen is the kernel's bottleneck at ~8-11ns/index/pair), and
    windows complete in-order so PE never builds a backlog.
  - pad slots carry idx -1: the Q7 desc-gen trims trailing negatives at
    runtime, so padding costs no descriptors or DMA bytes.  True counts
    are fed per-core via num_idxs_reg registers (batched reg_loads, and
    a no-sync total-order chain keeps Tile from reordering the stream).
  - per window one bulk DVE tensor_tensor is_equal with broadcast APs
    builds all T one-hots [128 edge, T*128 dst] at once (prebuilt up to
    16 windows ahead); TensorE accumulates aggT[din, dst] += Xchunk^T @
    onehot into PSUM (bf16 matmuls, f32 accumulate).
  - epilogue (PSUM->SBUF bf16 cast + recip scale on the Scalar engine,
    out matmul deferred 4 windows): out = (aggT^T @ WvT + deg x bv) *
    recip[dst].
"""

import os
import numpy as np

P = 128
NCORES = 8
N = 50000
XLO = 25088                   # rows in the low half of x (< 32768 for int16)
XHI = N - XLO
DIN = 128
DOUT = 128
WPC = 49                      # windows per core
NWIN = NCORES * WPC           # 392
NPAD = NWIN * P               # 50176
G = 1                         # windows per gather group

XG_BUFS = 12                  # gather-tile pipeline depth (windows in flight)

_last_exec_ns = None
_cache = {}


def _groups():
    out = []
    g0 = 0
    while g0 < WPC:
        out.append((g0, min(G, WPC - g0)))
        g0 += G
    return out


def _ensure_ntff_hook():
    """The agent image's ``antenv`` lacks ``axon_hooks``; provide the tiny
    get/set registry and register the ctypes NTFF hook so trace=True works."""
    import sys
    import types
    if "antenv.axon_hooks" in sys.modules:
        return
    try:
        import antenv
        mod = types.ModuleType("antenv.axon_hooks")
        _h = [None]
        mod.set_axon_ntff_profile_hook = lambda hook: _h.__setitem__(0, hook)
        mod.get_axon_ntff_profile_hook = lambda: _h[0]
        sys.modules["antenv.axon_hooks"] = mod
        antenv.axon_hooks = mod
        from trn_agent_boot.trn_boot import _ntff_profile_via_ctypes
        hook = _ntff_profile_via_ctypes("/opt/axon/libaxon_pjrt.so")
        if hook is not None:
            mod.set_axon_ntff_profile_hook(hook)
    except Exception:
        pass


NSEG = 4                      # sub-gathers per window (one per SWDGE queue)


def _segments(T_LO, T_HI):
    """Chunk ranges of the four per-window sub-gathers: (half, c0, c1).

    ceil split (5/4 chunks): measured FASTER than the count-balanced 4/5
    split despite a larger max segment — the two lightly-loaded queues
    finish early and the remaining pairs then run at lower Q7/SBUF
    contention, which beats equalized loads (A/B'd on HW, ~10-15us)."""
    sl = (T_LO + 1) // 2
    sh = (T_HI + 1) // 2
    return [(0, 0, sl), (0, sl, T_LO), (1, 0, sh), (1, sh, T_HI)]


def _offsets(T):
    """Column offsets of the packed [P, CW] f32 constant tensor.
    bf16 regions (wvt/bv/deg) occupy half-width f32 column spans."""
    o = {}
    o["cnt"] = 0                          # int32 true idx counts, WPC*NSEG
    o["idx16"] = o["cnt"] + WPC * NSEG    # int16 idx (wrapped), WPC*T*4 f32
    o["col"] = o["idx16"] + WPC * T * 4   # col_local f32, WPC*T cols
    o["rec"] = o["col"] + WPC * T         # recip, WPC cols
    o["wvt"] = o["rec"] + WPC             # Wv.T bf16, DOUT/2 f32 cols
    o["iota"] = o["wvt"] + DOUT // 2      # iota ramp 0..127, P cols
    o["bv"] = o["iota"] + P               # bv bf16 at partition 0
    o["deg"] = o["bv"] + DOUT // 2        # deg bf16 at partition 0
    o["CW"] = o["deg"] + WPC * P // 2
    return o


def _patch_qaware_dmasw_lanes():
    """Tile's DMASW semaphore-lane round-robin ignores dma_gather's
    queue_num, but a DMASW lane must only ever be fed from ONE SWDGE queue
    (per-queue FIFOs drain independently, so cross-queue sharing breaks the
    in-order wait accounting and CoreSim rejects it).  Partition the 8 lanes
    as 2 per queue: Pool-engine DMAs with queue_num q round-robin over lanes
    {2q, 2q+1}."""
    import concourse.tile_sem_assignment as tsa
    import concourse.mybir as mybir

    if getattr(tsa, "_gat_qaware_patch", False):
        return
    tsa._gat_qaware_patch = True
    orig = tsa.TileClockTick._assign_tick
    DMAInst = tsa.DMAInst

    def _assign_tick_qaware(self, inst):
        q = getattr(inst, "queue_num", None)
        if (
            q is not None
            and isinstance(inst, DMAInst)
            and inst.engine == mybir.EngineType.Pool
            and self.swdge_sem_count >= 8
        ):
            rr = getattr(self, "_gat_q_lane_rr", None)
            if rr is None:
                rr = self._gat_q_lane_rr = {}
            k = rr.get(q, 0)
            lanes_per_q = self.swdge_sem_count // 4
            self.next_sw_dma_idx = q * lanes_per_q + (k % lanes_per_q)
            rr[q] = k + 1
        return orig(self, inst)

    tsa.TileClockTick._assign_tick = _assign_tick_qaware


def _build(T, T_LO, T_HI):
    import concourse.bacc as bacc
    import concourse.mybir as mybir
    from concourse.tile import TileContext

    _patch_qaware_dmasw_lanes()

    f32 = mybir.dt.float32
    bf16 = mybir.dt.bfloat16
    i16 = mybir.dt.int16

    o = _offsets(T)
    CW = o["CW"]

    i32 = mybir.dt.int32

    nc = bacc.Bacc(None, target_bir_lowering=False, num_swdge_queues=4)
    xlo_d = nc.dram_tensor("xlo", [XLO, DIN], bf16, kind="ExternalInput")
    xhi_d = nc.dram_tensor("xhi", [XHI, DIN], bf16, kind="ExternalInput")
    # int32 (not f32): the idx16 region's -1 padding forms NaN bit patterns
    # that float-dtype DMA validation would reject
    const_d = nc.dram_tensor("const", [P, CW], i32, kind="ExternalInput")
    out_d = nc.dram_tensor("out", [WPC * P, DOUT], f32, kind="ExternalOutput")

    with TileContext(nc) as tc:
        with (
            tc.tile_pool(name="const", bufs=1) as cpool,
            tc.tile_pool(name="xg", bufs=XG_BUFS) as xgpool,
            tc.tile_pool(name="oh", bufs=16) as ohpool,
            tc.tile_pool(name="ep", bufs=7) as eppool,
            tc.tile_pool(name="ps", bufs=3, space="PSUM") as pspool,
            tc.tile_pool(name="po", bufs=4, space="PSUM") as popool,
            tc.tile_pool(name="wp", bufs=1, space="PSUM") as wpool,
        ):
            const_sb = cpool.tile([P, CW], i32)
            # cnt + first 8 windows' idx16 first so the first gathers start
            # almost immediately; everything else afterwards
            idx_split = o["idx16"] + 8 * T * 8
            nc.sync.dma_start(out=const_sb[:, 0:idx_split],
                              in_=const_d[:, 0:idx_split])
            nc.sync.dma_start(out=const_sb[:, idx_split:o["col"]],
                              in_=const_d[:, idx_split:o["col"]])
            nc.sync.dma_start(out=const_sb[:, o["col"]:],
                              in_=const_d[:, o["col"]:])

            cnt_sb = const_sb[:, o["cnt"]:o["idx16"]]
            idx16_sb = const_sb[:, o["idx16"]:o["col"]].bitcast(i16)
            col_sb = const_sb[:, o["col"]:o["col"] + WPC * T].bitcast(f32)
            rec_sb = const_sb[:, o["rec"]:o["rec"] + WPC].bitcast(f32)
            wvt_sb = const_sb[:, o["wvt"]:o["wvt"] + DOUT // 2].bitcast(bf16)
            iota_sb = const_sb[:, o["iota"]:o["iota"] + P].bitcast(f32)
            bv_sb = const_sb[0:1, o["bv"]:o["bv"] + DOUT // 2].bitcast(bf16)
            deg_sb = const_sb[0:1, o["deg"]:o["deg"] + WPC * P // 2].bitcast(bf16)

            warm_ps = wpool.tile([1, 1], f32, tag="warm")
            # PE observes the const-load semaphore once
            cw0 = const_sb[0:1, 0:1].bitcast(f32)
            nc.tensor.matmul(out=warm_ps[:], lhsT=cw0,
                             rhs=cw0, start=True, stop=True)

            EPI_DEFER = 4
            pending = []

            def _epilogue(w, aggT_sb):
                out_ps = popool.tile([P, DOUT], f32, tag="outp")
                nc.tensor.matmul(out=out_ps[:], lhsT=aggT_sb[:],
                                 rhs=wvt_sb[:], start=True, stop=False)
                nc.tensor.matmul(out=out_ps[:],
                                 lhsT=deg_sb[0:1, w * P:(w + 1) * P],
                                 rhs=bv_sb[0:1, :], start=False, stop=True)
                out_sb = eppool.tile([P, DOUT], f32, tag="outs")
                nc.scalar.activation(out=out_sb[:], in_=out_ps[:],
                                     func=mybir.ActivationFunctionType.Copy,
                                     scale=rec_sb[:, w:w + 1])
                nc.sync.dma_start(out=out_d[w * P:(w + 1) * P, :],
                                  in_=out_sb[:])

            segs = _segments(T_LO, T_HI)
            # 4 windows per register bank: one batched TENSOR_LOAD fills 16
            # count registers, keeping the Pool issue queue packed with
            # gathers (in-flight depth across the 4 Q7 pairs)
            RLW = 4
            cregs = [nc.gpsimd.alloc_register(f"cntreg{q}")
                     for q in range(RLW * NSEG)]
            # Tile's scheduler doesn't track register data deps, and the
            # emitted Pool-stream ORDER determines both register-read
            # correctness (num_idxs_reg resolves at NX decode, in issue
            # order) and Q7-pair overlap (adjacent instructions must hit
            # different queues).  Freeze the whole stream with a
            # total-order no-sync chain: rl -> g(q0) -> g(q1) -> ...
            from concourse.instruction_name_ordered_set import (
                InstructionNameOrderedSet)
            chain_prev = [None]

            def _chain(inst):
                if chain_prev[0] is not None:
                    deps = InstructionNameOrderedSet()
                    deps.add(chain_prev[0].ins.name)
                    inst.ins.add_nosync_dependencies_from(deps)
                chain_prev[0] = inst

            def _load_counts(w0, nwin):
                regs = cregs[:nwin * NSEG]
                rl = nc.gpsimd.reg_load(
                    regs, cnt_sb[0:1, w0 * NSEG:(w0 + nwin) * NSEG])
                _chain(rl)

            def _chain_gather(g):
                _chain(g)

            goff16 = 0
            for gidx, (g0, Gg) in enumerate(_groups()):
                # every window splits into NSEG sub-gathers, one per SWDGE
                # queue: all four Q7 core-pairs work the same window in
                # lock-step, so windows complete in order and PE never
                # builds a backlog
                w = g0
                xg = xgpool.tile([P, Gg * T * P], bf16, tag="xg")
                if gidx < XG_BUFS:
                    # zero each slot's first use: trailing-trimmed gathers
                    # leave pad slots holding whatever SBUF held before, and
                    # NaN bit patterns would poison 0-weighted matmuls
                    nc.vector.memset(xg[:], 0)
                xg3 = xg[:].rearrange("p (c e) -> p c e", e=P)
                if gidx % RLW == 0:
                    _load_counts(w, min(RLW, WPC - w))
                for s, (half, c0, c1) in enumerate(segs):
                    ni = (c1 - c0) * P
                    cbase = 0 if half == 0 else T_LO
                    src = xlo_d if half == 0 else xhi_d
                    g = nc.gpsimd.dma_gather(
                        out_ap=xg3[:, cbase + c0:cbase + c1, :],
                        in_ap=src[:, :],
                        idxs_ap=idx16_sb[:, goff16:goff16 + ni // 16],
                        num_idxs=ni,
                        num_idxs_reg=cregs[(gidx % RLW) * NSEG + s],
                        elem_size=DIN,
                        single_packet=False,
                        queue_num=s,
                    )
                    _chain_gather(g)
                    goff16 += ni // 16
                warm_ps = wpool.tile([1, 1], f32, tag="warm")
                # PE observes the gather completions here
                nc.tensor.matmul(out=warm_ps[:], lhsT=xg[0:1, 0:1],
                                 rhs=xg[0:1, 0:1], start=True, stop=True)
                for wl in range(Gg):
                    w = g0 + wl
                    # bulk one-hot: oh[p, t*128+j] = (col[p, w*T+t] == j)
                    oh = ohpool.tile([P, T * P], bf16, tag="oh")
                    oh3 = oh[:].rearrange("p (t j) -> p t j", j=P)
                    iota_b = iota_sb[:, :].unsqueeze(1).to_broadcast((P, T, P))
                    col_b = col_sb[:, w * T:(w + 1) * T].unsqueeze(2) \
                        .to_broadcast((P, T, P))
                    nc.vector.tensor_tensor(
                        out=oh3, in0=iota_b, in1=col_b,
                        op=mybir.AluOpType.is_equal,
                    )
                    agg_ps = pspool.tile([P, P], f32, tag="agg")
                    for t in range(T):
                        if t < T_LO:
                            c = wl * T_LO + t
                        else:
                            c = Gg * T_LO + wl * T_HI + (t - T_LO)
                        nc.tensor.matmul(
                            out=agg_ps[:],
                            lhsT=xg[:, c * P:(c + 1) * P],
                            rhs=oh[:, t * P:(t + 1) * P],
                            start=(t == 0),
                            stop=(t == T - 1),
                        )
                    # the first epilogue half (PSUM->SBUF cast on ACT) can
                    # chase the accumulation immediately
                    aggT_sb = eppool.tile([P, P], bf16, tag="aggT")
                    nc.scalar.activation(out=aggT_sb[:], in_=agg_ps[:],
                                         func=mybir.ActivationFunctionType.Copy)
                    # defer the PE half of the epilogue by EPI_DEFER windows
                    # so PE never blocks on the ACT round trip
                    pending.append((w, aggT_sb))
                    if len(pending) > EPI_DEFER:
                        _epilogue(*pending.pop(0))
            for args in pending:
                _epilogue(*args)
    nc.compile()
    return nc


def _prep(x, row, col):
    """Host-side packing. Returns (T, T_LO, T_HI, per-core arrays)."""
    row = row.astype(np.int64)
    col = col.astype(np.int64)
    E = len(row)
    ishi = (row >= XLO).astype(np.int64)
    key = ((col >> 7) << 1) | ishi
    order = np.argsort(key, kind="stable")
    srow = row[order]
    scol = col[order]
    skey = key[order]

    deg = np.bincount(col, minlength=NPAD).astype(np.float32)
    recip = np.where(deg > 0, 1.0 / np.maximum(deg, 1.0), 0.0).astype(np.float32)

    cnt = np.bincount(key, minlength=2 * NWIN)
    lo_cnt, hi_cnt = cnt[0::2], cnt[1::2]
    T_LO = int(np.ceil(lo_cnt.max() / P))
    T_HI = int(np.ceil(hi_cnt.max() / P))
    T = T_LO + T_HI

    gstart = np.zeros(2 * NWIN + 1, np.int64)
    np.cumsum(cnt, out=gstart[1:])
    epos = np.arange(E, dtype=np.int64) - gstart[skey]
    p = epos % P
    tw = epos // P
    whalf = skey & 1
    win = skey >> 1
    tchunk = np.where(whalf == 1, tw + T_LO, tw)

    col_arr = np.full((NWIN, P, T), -1.0, np.float32)
    col_arr[win, p, tchunk] = (scol & (P - 1)).astype(np.float32)

    # padding slots get idx -1: the Q7 desc-gen kernel trims trailing
    # negative indices at runtime (per core), skipping their descriptors
    # and DMA bytes.  Their one-hot cols are -1 so the stale SBUF data in
    # those slots never contributes to the matmul.
    pad = np.int16(0) if os.environ.get("GAT_SIM_NOTRIM") else np.int16(-1)
    idx_lo = np.full((NWIN, T_LO * P), pad, np.int16)
    idx_hi = np.full((NWIN, T_HI * P), pad, np.int16)
    lo_m = whalf == 0
    hi_m = whalf == 1
    idx_lo[win[lo_m], epos[lo_m]] = srow[lo_m].astype(np.int16)
    idx_hi[win[hi_m], epos[hi_m]] = (srow[hi_m] - XLO).astype(np.int16)

    segs = _segments(T_LO, T_HI)
    per_core = []
    for c in range(NCORES):
        wsl = slice(c * WPC, (c + 1) * WPC)
        # wrapped idx16 layout: per sub-gather, index i at [i%16, i//16],
        # replicated across the 8 groups of 16 partitions.  True counts
        # (non-pad indices) per sub-gather feed num_idxs_reg so the Q7
        # desc-gen's trailing-negative trim matches the decode bookkeeping.
        cols16 = []
        cnts = np.zeros(WPC * NSEG, np.int32)
        for wl in range(WPC):
            wabs = c * WPC + wl
            for s, (half, c0, c1) in enumerate(segs):
                arr = idx_lo if half == 0 else idx_hi
                havecnt = int(lo_cnt[wabs] if half == 0 else hi_cnt[wabs])
                flat = arr[wabs, c0 * P:c1 * P].copy()
                if os.environ.get("GAT_SIM_NOTRIM"):
                    cnt_s = (c1 - c0) * P    # pads are idx 0: all "valid"
                else:
                    cnt_s = min(max(havecnt - c0 * P, 0), (c1 - c0) * P)
                    if cnt_s == 0:
                        flat[0] = 0      # keep >=1 valid idx per sub-gather
                        cnt_s = 1
                cnts[wl * NSEG + s] = cnt_s
                wrapped = flat.reshape(-1, 16).T             # [16, ni/16]
                cols16.append(np.tile(wrapped, (8, 1)))      # [128, ni/16]
        idx16_map = np.concatenate(cols16, axis=1)           # [128, WPC*T*8]
        col_map = np.ascontiguousarray(
            col_arr[wsl].transpose(1, 0, 2).reshape(P, WPC * T))
        rec_map = np.ascontiguousarray(
            recip[c * WPC * P:(c + 1) * WPC * P].reshape(WPC, P).T)
        deg_map = np.ascontiguousarray(
            deg[c * WPC * P:(c + 1) * WPC * P].reshape(1, WPC * P))
        per_core.append((idx16_map, col_map, rec_map, deg_map, cnts))
    return T, T_LO, T_HI, per_core


def _pack_const(T, idx16_map, col_map, rec_map, deg_map, cnts, wvt_bf, bv_bf):
    """Pack the [P, CW] f32 const tensor.  bf16 payloads (wvt/bv/deg) are
    written through a uint16 view at doubled column offsets."""
    o = _offsets(T)
    arr = np.zeros((P, o["CW"]), np.float32)
    u16 = arr.view(np.uint16)
    assert idx16_map.shape == (P, WPC * T * 8)
    arr[0:1, o["cnt"]:o["idx16"]] = cnts.astype(np.int32).view(np.float32)
    arr[:, o["idx16"]:o["col"]] = idx16_map.view(np.float32)
    arr[:, o["col"]:o["col"] + WPC * T] = col_map
    arr[:, o["rec"]:o["rec"] + WPC] = rec_map
    u16[:, 2 * o["wvt"]:2 * o["wvt"] + DOUT] = wvt_bf.view(np.uint16)
    arr[:, o["iota"]:o["iota"] + P] = np.arange(P, dtype=np.float32)[None, :]
    u16[0, 2 * o["bv"]:2 * o["bv"] + DOUT] = bv_bf.view(np.uint16).ravel()
    u16[0, 2 * o["deg"]:2 * o["deg"] + WPC * P] = \
        deg_map.astype(wvt_bf.dtype).view(np.uint16).ravel()
    return arr


def _device_inputs(inputs):
    """Host prep shared by kernel() and the sim harness.
    Returns (T, T_LO, T_HI, in_maps)."""
    import concourse.mybir as mybir
    bf16 = mybir.dt.np(mybir.dt.bfloat16)

    x = np.ascontiguousarray(np.asarray(inputs["x"], dtype=np.float32))
    ei = np.asarray(inputs["edge_index"])
    row = np.asarray(ei[0]).astype(np.int64)
    col = np.asarray(ei[1]).astype(np.int64)
    Wv = np.asarray(inputs["Wv"], dtype=np.float32)
    bv = np.asarray(inputs["bv"], dtype=np.float32)

    wvt_bf = np.ascontiguousarray(Wv.T.astype(bf16))       # [DIN, DOUT] bf16
    bv_bf = np.ascontiguousarray(bv.reshape(1, DOUT).astype(bf16))

    T, T_LO, T_HI, per_core = _prep(x, row, col)

    xbf = x.astype(bf16)
    xlo = np.ascontiguousarray(xbf[:XLO])
    xhi = np.ascontiguousarray(xbf[XLO:])
    in_maps = []
    for c in range(NCORES):
        const = _pack_const(T, *per_core[c], wvt_bf, bv_bf)
        in_maps.append({"xlo": xlo, "xhi": xhi,
                        "const": const.view(np.int32)})
    return T, T_LO, T_HI, in_maps


def kernel(**inputs):
    global _last_exec_ns
    _ensure_ntff_hook()
    from concourse.bass_utils import run_bass_kernel_spmd

    T, T_LO, T_HI, in_maps = _device_inputs(inputs)

    key = (T, T_LO, T_HI)
    if key not in _cache:
        _cache[key] = _build(T, T_LO, T_HI)
    nc = _cache[key]

    trace = bool(os.environ.get("GAT_TRACE"))
    res = run_bass_kernel_spmd(nc, in_maps, list(range(NCORES)), trace=trace)
    _last_exec_ns = res.exec_time_ns
    globals()["_last_res"] = res

    out = np.concatenate([res.results[c]["out"] for c in range(NCORES)], axis=0)
    return np.ascontiguousarray(out[:N])
